# revision 32
# baseline (speedup 1.0000x reference)
"""Trainium2 Bass kernel for masked multi-head attention + depthwise residual conv.

Reference computation (per batch b):
    qkv = x @ W_qkv -> split (3, heads=8, d=64)
    dots = q @ k^T * d**-0.5 ; key-masked softmax
    out  = attn @ v + depthwise_conv33_seq(v)
    out  = out @ W_out + b_out ; row-masked to 0

Sharding: 16 (batch, head-pair) units -> 8 cores, each core handles one batch
and two adjacent heads, producing a partial [2048, 512] projection output
(bf16). Host sums the 4 partials per batch, adds b_out, applies the row mask.

Device-side layout: dots are computed transposed, dotsT[j, i] (keys on
partitions), so the key mask folds into the exp bias and softmax needs no
max-pass. exp writes fp8 attention numerators with a -2 bias shift (cancels
in softmax); attn@v runs in fp8e4m3 DoubleRow, contracting key-chunk pairs.
The two heads' attn@v accumulators land on disjoint PSUM partitions
(h0 -> 0-63 + l0@64, h1 -> 64-127 + l1@32 via lhsT column placement), so the
output projection contracts both heads in one 128-deep matmul per i-block.
v is produced directly in [j, (h,d)] layout (x-chunk as stationary operand),
which also feeds the depthwise conv: a banded-Toeplitz product against three
tiny shift-invariant band templates (main 128x128 + two 16x128 edge blocks)
instead of full Toeplitz blocks - saves 1.5MB of input DMA.

The emission order software-pipelines dots+exp ahead of attn@v so the Act
engine (the exp bottleneck, ~1.1us per key-chunk) never starves: input DMA
issues wqk + the first x columns first, and every attn@v step tops up one
dots pair for the next chunk.
"""

import sys

sys.path.insert(0, "/opt/trn_rl_repo")

from contextlib import ExitStack

import numpy as np

import concourse.bass as bass
import concourse.tile as tile
from concourse import bacc, mybir

F32 = mybir.dt.float32
BF16 = mybir.dt.bfloat16
FP8 = mybir.dt.float8e4
DR = mybir.MatmulPerfMode.DoubleRow

HEADS = 8
D = 64
DIM = 512
KER = 33
PAD = KER // 2
SCALE = D ** -0.5
B = 2
N = 2048
NCORES = 8
NEG = -1.0e30


def _build_body(ctx: ExitStack, tc: "tile.TileContext", ins, outs, dbg=None):
    nc = tc.nc
    xT, wqk, wv, wout, tmain, tedge, mbias = ins
    out = outs[0]

    Exp = mybir.ActivationFunctionType.Exp

    singles = ctx.enter_context(tc.tile_pool(name="singles", bufs=1))
    ptpool = ctx.enter_context(tc.tile_pool(name="ptpool", bufs=5))
    lpool = ctx.enter_context(tc.tile_pool(name="lpool", bufs=2))
    opool = ctx.enter_context(tc.tile_pool(name="opool", bufs=2))
    # PSUM banks: psA 2x4KB (dots pairs + qk/vdir staging), psAV 2x2KB (the
    # two attn@v accumulators), psP 2x2KB ring shared by conv/bc/proj.
    psA = ctx.enter_context(tc.tile_pool(name="psA", bufs=2, space="PSUM"))
    psAV = ctx.enter_context(tc.tile_pool(name="psAV", bufs=2, space="PSUM"))
    psP = ctx.enter_context(tc.tile_pool(name="psP", bufs=2, space="PSUM"))

    # ---- resident SBUF tensors; DMA order = need order ----
    xt_sb = singles.tile([128, 4, N], BF16)  # xT chunks: [p, kc, i]
    wqk_sb = singles.tile([128, 4, 256], BF16)  # [p, kc, fc*128+(h,d)]
    mb_sb = singles.tile([128, 16], F32)  # mask bias per j: [p, jc]
    wv_sb = singles.tile([128, 4, 128], BF16)  # [p, kc, (h,d)]
    wout_sb = singles.tile([128, 512], BF16)  # [(h,d), f]
    tm_sb = singles.tile([128, 2, 128], BF16)  # conv main band: [pj, h, fi]
    te_sb = singles.tile([128, 2, 2, 128], BF16)  # conv edges: [pj, up/dn, h, fi]

    # Input DMA split across both HWDGE rings (sync + scalar) so the first
    # x columns and qk weights land in parallel; the Act engine is idle here.
    xTr = xT.rearrange("(c p) i -> p c i", p=128)
    nc.sync.dma_start(out=wqk_sb[:, :, 128:256], in_=wqk[:, :, 128:256])
    nc.scalar.dma_start(out=xt_sb[:, :, 0:512], in_=xTr[:, :, 0:512])
    nc.sync.dma_start(out=wqk_sb[:, :, 0:128], in_=wqk[:, :, 0:128])
    nc.sync.dma_start(out=mb_sb[:], in_=mbias[:, :])
    nc.sync.dma_start(out=wv_sb[:], in_=wv)
    nc.scalar.dma_start(out=xt_sb[:, :, 512:1024], in_=xTr[:, :, 512:1024])
    nc.sync.dma_start(out=xt_sb[:, :, 1024:1536], in_=xTr[:, :, 1024:1536])
    nc.scalar.dma_start(out=xt_sb[:, :, 1536:2048], in_=xTr[:, :, 1536:2048])
    nc.sync.dma_start(out=tm_sb[:], in_=tmain)
    nc.sync.dma_start(out=te_sb[:], in_=tedge)
    nc.sync.dma_start(out=wout_sb[:], in_=wout)

    onesb = singles.tile([128, 64], BF16)
    nc.gpsimd.memset(onesb[:], 1.0)
    # PE warmup: ~3.4us of matmuls on dummy data while the input DMA runs, so
    # the HAM clock gate reaches 2.4GHz before the first real matmul.
    warmsb = singles.tile([128, 512], BF16)
    nc.vector.memset(warmsb[:], 0.5)
    warmps = psA.tile([128, 512], F32, tag="A")
    for i in range(16):
        nc.tensor.matmul(
            warmps[:], warmsb[:, 0:128], warmsb[:], start=(i == 0), stop=(i == 15)
        )
    nc.vector.tensor_copy(warmsb[0:1, 0:1], warmps[0:1, 0:1])
    # v fp8 for attn@v: [j, jc, 256] cols = [d0 | ones@64 | 0 | ones1@160 | d1@192]
    v_sb = singles.tile([128, 16, 256], FP8, name="v_sb")
    nc.gpsimd.memset(v_sb[:], 0.0)
    nc.gpsimd.memset(v_sb[:, :, 64], 1.0)
    nc.gpsimd.memset(v_sb[:, :, 160], 1.0)

    v_cv = singles.tile([128, 16, 2, 64], BF16)  # conv v: [j, jc, h, d]
    qt_sb = singles.tile([128, N], BF16)  # qT: rows h*64+d, cols i
    kt_sb = singles.tile([128, N], BF16)
    projin = singles.tile([128, 16, 128], BF16)  # [(h,d), iblk, i]

    # ---- q/k projections (bf16) ----
    def emit_qk(fc, ic):
        ps = psP.tile([128, 512], F32, tag="P")
        for kc in range(4):
            nc.tensor.matmul(
                ps[:],
                wqk_sb[:, kc, fc * 128 : (fc + 1) * 128],
                xt_sb[:, kc, ic * 512 : (ic + 1) * 512],
                start=(kc == 0),
                stop=(kc == 3),
            )
        dst = qt_sb if fc == 0 else kt_sb
        nc.vector.tensor_copy(dst[:, ic * 512 : (ic + 1) * 512], ps[:])

    # ---- v directly in [j, (h,d)] layout: x-block as stationary operand ----
    def emit_vdir(jb):
        ps = psP.tile([128, 128], F32, tag="P")
        for kc in range(4):
            nc.tensor.matmul(
                ps[:],
                xt_sb[:, kc, jb * 128 : (jb + 1) * 128],
                wv_sb[:, kc, :],
                start=(kc == 0),
                stop=(kc == 3),
            )
        nc.vector.tensor_copy(v_cv[:, jb, :, :], ps[:])
        # fp8 copy into attn@v lhsT slots: h0 -> cols 0:64, h1 -> cols 192:256
        dst = v_sb[:, jb, :].rearrange("p (four s) -> p four s", s=64)
        nc.gpsimd.tensor_copy(dst[:, 0:4:3, :], v_cv[:, jb, :, :])

    # ---- attention: dots -> exp(fp8) -> attn@v (DR over jc pairs) ----
    def emit_pair(ic, p):
        """dots + exp for key chunks (2p, 2p+1) of chunk ic; returns pt2."""
        pt2 = ptpool.tile([128, 2, 2, 512], FP8, tag="pt")  # [j, h, mem, i]
        i5 = slice(ic * 512, (ic + 1) * 512)
        for mem in range(2):
            jc = 2 * p + mem
            dots = psA.tile([128, 2, 512], F32, tag="A")
            for h in range(2):
                nc.tensor.matmul(
                    dots[:, h, :],
                    kt_sb[h * 64 : (h + 1) * 64, jc * 128 : (jc + 1) * 128],
                    qt_sb[h * 64 : (h + 1) * 64, i5],
                    start=True,
                    stop=True,
                )
            nc.scalar.activation(
                pt2[:, :, mem, :],
                dots[:],
                Exp,
                bias=mb_sb[:, jc : jc + 1],
                scale=SCALE,
            )
        return pt2

    def emit_conv(ic, ks):
        """Banded conv for i-blocks ks of chunk ic into the shared psum tile.
        res[h*64+d, k, i] = sum_j v[j, (h,d)] * w_h[j - i + 16]."""
        if 0 in ks:
            emit_conv.rps = psP.tile([128, 4, 128], F32, tag="P")
        rps = emit_conv.rps
        for k in ks:
            kk = 4 * ic + k
            for h in range(2):
                o = rps[h * 64 : (h + 1) * 64, k, :]
                nc.tensor.matmul(
                    o, v_cv[:, kk, h, :], tm_sb[:, h, :],
                    start=True, stop=not (kk > 0 or kk < 15),
                )
                if kk > 0:
                    nc.tensor.matmul(
                        o, v_cv[:, kk - 1, h, :], te_sb[:, 0, h, :],
                        start=False, stop=not (kk < 15),
                    )
                if kk < 15:
                    nc.tensor.matmul(
                        o, v_cv[:, kk + 1, h, :], te_sb[:, 1, h, :],
                        start=False, stop=True,
                    )

    def conv_finish(ic):
        rsb = lpool.tile([128, 4, 128], BF16, tag="res")
        nc.vector.tensor_copy(rsb[:], emit_conv.rps[:])
        return rsb

    Copy = mybir.ActivationFunctionType.Copy

    def emit_proj(ic, subs=(0, 1, 2, 3), final=False):
        """Merged head projection of chunk ic + output DMA (bf16). In the
        final call the Act engine is idle, so casts alternate vector/scalar."""
        if 0 in subs:
            emit_proj.osb = opool.tile([128, 4, DIM], BF16, tag="osb")
        osb = emit_proj.osb
        outr = out.rearrange("(ic c p) f -> ic p c f", c=4, p=128)[ic]
        for sub in subs:
            pp = psP.tile([128, 512], F32, tag="P")
            nc.tensor.matmul(
                pp[:], projin[:, 4 * ic + sub, :], wout_sb[:],
                start=True, stop=True,
            )
            if final:
                nc.scalar.activation(osb[:, sub, :], pp[:], Copy)
            else:
                nc.vector.tensor_copy(osb[:, sub, :], pp[:])
            nc.sync.dma_start(out=outr[:, sub, :], in_=osb[:, sub, :])

    # ---- software pipeline ----
    # The Act engine (exp: ~2.2us per dots pair, 32 pairs) is the critical
    # engine; everything else interleaves under its shadow. Pairs are emitted
    # at most ~3 ahead of their attn@v consumer so the 2-deep dots PSUM ring
    # never stalls the in-order PE stream; qk/vdir/conv/proj slot into the
    # gaps just-in-time.
    emit_qk(1, 0)
    emit_qk(0, 0)
    pt2s = {0: emit_pair(0, 0)}
    for jb in range(4):
        emit_vdir(jb)
    emit_qk(1, 1)
    pt2s[1] = emit_pair(0, 1)

    extras = {
        0: [lambda: emit_qk(0, 1), lambda: emit_vdir(4), lambda: emit_vdir(5)],
        1: [lambda: emit_qk(1, 2), lambda: emit_vdir(6), lambda: emit_vdir(7)],
        2: [lambda: emit_vdir(8), lambda: emit_vdir(9)],
        3: [lambda: emit_qk(1, 3), lambda: emit_vdir(10), lambda: emit_vdir(11)],
        4: [lambda: emit_qk(0, 2), lambda: emit_vdir(12), lambda: emit_vdir(13)],
        5: [lambda: emit_qk(0, 3), lambda: emit_vdir(14), lambda: emit_vdir(15)],
    }
    g = 2
    avh = None
    res = None
    def emit_attnv(s):
        ic, p = s // 8, s % 8
        pt2 = pt2s.pop(s)
        for h in range(2):
            nc.tensor.matmul(
                avh[h][:],
                v_sb[:, 2 * p : 2 * p + 2, 128 * h : 128 * h + 128],
                pt2[:, h, :, :],
                start=(p == 0),
                stop=(p == 7),
                perf_mode=DR,
            )

    for s in range(32):
        ic, p = s // 8, s % 8
        lim = min(32, s + 4 if p == 7 else s + 3)
        if p == 0:
            # pairs first: the fresh avh allocation below may stall on the
            # previous chunk's epilogue, and dots behind it would starve Act
            while g < lim:
                pt2s[g] = emit_pair(g // 8, g % 8)
                g += 1
            avh = [psAV.tile([128, 4, 128], F32, tag="AV", name="av") for _ in range(2)]
            emit_attnv(s)
        else:
            # attnv first: its exp dependency is older than the new pairs',
            # so the epilogue (at p==7) isn't delayed behind fresh dots
            emit_attnv(s)
            while g < lim:
                pt2s[g] = emit_pair(g // 8, g % 8)
                g += 1
        for fn in extras.get(s, ()):
            fn()
        if p == 2 and ic >= 1:
            emit_proj(ic - 1, (0, 1))
        if p == 3 and ic >= 1:
            emit_proj(ic - 1, (2, 3))
        if p == 4:
            emit_conv(ic, (0,))
        if p == 5:
            emit_conv(ic, (1,))
        if p == 6:
            emit_conv(ic, (2, 3))
            res = conv_finish(ic)
        if p < 7:
            continue

        # epilogue: l rows -> broadcast -> rc=1/l -> projin (heads merged)
        l_sb = lpool.tile([128, 4, 128], BF16, tag="l")
        nc.vector.tensor_copy(l_sb[64:65, :, :], avh[0][64:65, :, :])
        if ic == 3:  # Act engine is idle after the last exp
            nc.scalar.activation(l_sb[32:33, :, :], avh[1][32:33, :, :], Copy)
        else:
            nc.vector.tensor_copy(l_sb[32:33, :, :], avh[1][32:33, :, :])
        bc = psP.tile([128, 4, 128], F32, tag="P")
        nc.tensor.matmul(
            bc[0:64, :, :], onesb[64:65, :], l_sb[64:65, :, :],
            start=True, stop=True,
        )
        nc.tensor.matmul(
            bc[64:128, :, :], onesb[32:33, :], l_sb[32:33, :, :],
            start=True, stop=True,
        )
        # rc/mul/add split along the free dim (sub-block halves) so the final
        # projection can start on the first half while the second half's DVE
        # ops are still running.
        rc = lpool.tile([128, 4, 128], F32, tag="rc")
        pslice = projin[:, 4 * ic : 4 * ic + 4, :]
        if ic < 3:
            nc.vector.reciprocal_approx_fast(rc[:], bc[:])
            nc.vector.tensor_mul(pslice[0:64], avh[0][0:64, :, :], rc[0:64])
            nc.vector.tensor_mul(pslice[64:128], avh[1][64:128, :, :], rc[64:128])
            nc.vector.tensor_add(pslice[:], pslice[:], res[:])
        else:
            # final chunk: halve the epilogue along the free dim and start the
            # projection on the first half early; casts go to the idle Act
            # engine so the DVE chain is never blocked behind a PE wait.
            for half in range(2):
                hs = slice(2 * half, 2 * half + 2)
                nc.vector.reciprocal_approx_fast(rc[:, hs, :], bc[:, hs, :])
                nc.vector.tensor_mul(
                    pslice[0:64, hs, :], avh[0][0:64, hs, :], rc[0:64, hs, :]
                )
                nc.vector.tensor_mul(
                    pslice[64:128, hs, :], avh[1][64:128, hs, :], rc[64:128, hs, :]
                )
                nc.vector.tensor_add(
                    pslice[:, hs, :], pslice[:, hs, :], res[:, hs, :]
                )
                emit_proj(3, (0, 1) if half == 0 else (2, 3), final=True)
        if dbg is not None and ic == 0:
            asb = singles.tile([128, 4, 128], F32, tag="dbgav", name="dbgav")
            nc.vector.tensor_copy(asb[:], avh[0][:])
            nc.sync.dma_start(out=dbg["av0"], in_=asb[:])
    if dbg is not None:
        nc.sync.dma_start(out=dbg["qt"], in_=qt_sb[:])
        nc.sync.dma_start(out=dbg["kt"], in_=kt_sb[:])
        nc.sync.dma_start(out=dbg["v0"], in_=v_sb[:])

    if dbg is not None:
        nc.sync.dma_start(out=dbg["projin"], in_=projin[:])


_NC_CACHE = {}


def _dram_tensors(nc):
    ins = [
        nc.dram_tensor("xT", [DIM, N], BF16, kind="ExternalInput").ap(),
        nc.dram_tensor("wqk", [128, 4, 256], BF16, kind="ExternalInput").ap(),
        nc.dram_tensor("wv", [128, 4, 128], BF16, kind="ExternalInput").ap(),
        nc.dram_tensor("wout", [128, 512], BF16, kind="ExternalInput").ap(),
        nc.dram_tensor("tmain", [128, 2, 128], BF16, kind="ExternalInput").ap(),
        nc.dram_tensor("tedge", [128, 2, 2, 128], BF16, kind="ExternalInput").ap(),
        nc.dram_tensor("mbias", [128, 16], F32, kind="ExternalInput").ap(),
    ]
    outs = [nc.dram_tensor("out", [N, DIM], BF16, kind="ExternalOutput").ap()]
    return ins, outs


def _get_nc(reps: int = 1):
    if reps in _NC_CACHE:
        return _NC_CACHE[reps]
    nc = bacc.Bacc(
        "TRN2",
        target_bir_lowering=False,
        debug=False,
        num_devices=NCORES,
    )
    ins, outs = _dram_tensors(nc)
    with tile.TileContext(nc) as tc:
        if reps == 1:
            with ExitStack() as ctx:
                _build_body(ctx, tc, ins, outs)
        else:
            with tc.For_i(0, reps, 1):
                with ExitStack() as ctx:
                    _build_body(ctx, tc, ins, outs)
    nc.compile()
    _NC_CACHE[reps] = nc
    return nc


def _get_nc_debug():
    nc = bacc.Bacc(
        "TRN2", target_bir_lowering=False, debug=False, num_devices=NCORES
    )
    ins, outs = _dram_tensors(nc)
    dbg = {
        "qt": nc.dram_tensor("d_qt", [128, N], BF16, kind="ExternalOutput").ap(),
        "kt": nc.dram_tensor("d_kt", [128, N], BF16, kind="ExternalOutput").ap(),
        "v0": nc.dram_tensor("d_v0", [128, 16, 256], FP8, kind="ExternalOutput").ap(),
        "av0": nc.dram_tensor("d_av0", [128, 4, 128], F32, kind="ExternalOutput").ap(),
        "projin": nc.dram_tensor(
            "d_projin", [128, 16, 128], BF16, kind="ExternalOutput"
        ).ap(),
    }
    with tile.TileContext(nc) as tc:
        with ExitStack() as ctx:
            _build_body(ctx, tc, ins, outs, dbg=dbg)
    nc.compile()
    return nc


def _conv_templates(conv_w_pair: np.ndarray):
    """[2, 33] taps -> shift-invariant band blocks for 128-wide i-blocks.

    tmain[p, h, fi] = w_h[p - fi + 16]        (same j-block)
    tup[p, h, fi]   = w_h[p - 112 - fi]       (prev block, rows 112-127 live)
    tdn[p, h, fi]   = w_h[144 + p - fi]       (next block, rows 0-15 live)
    """
    tmain = np.zeros((128, 2, 128), np.float32)
    tedge = np.zeros((128, 2, 2, 128), np.float32)
    pj = np.arange(128)[:, None]
    fi = np.arange(128)[None, :]
    for h in range(2):
        w = conv_w_pair[h]
        idx = pj - fi + PAD
        valid = (idx >= 0) & (idx < KER)
        tmain[:, h, :][valid] = w[idx[valid]]
        iu = pj - 112 - fi
        vu = (iu >= 0) & (iu < KER)
        tedge[:, 0, h, :][vu] = w[iu[vu]]
        idn = 144 + pj - fi
        vd = (idn >= 0) & (idn < KER)
        tedge[:, 1, h, :][vd] = w[idn[vd]]
    return tmain, tedge


def _make_in_maps(x, mask, W_qkv, W_out, conv_w):
    import ml_dtypes

    x = np.asarray(x, np.float32)
    mask = np.asarray(mask)
    W_qkv = np.asarray(W_qkv, np.float32)
    W_out = np.asarray(W_out, np.float32)
    conv_w = np.asarray(conv_w, np.float32)

    bf = ml_dtypes.bfloat16
    c128 = np.arange(128)
    in_maps = []
    for core in range(NCORES):
        b = core // 4
        h0 = (core % 4) * 2
        # wqk: [p, kc, fc*128+c] with c=(h*64+d)
        wqk_cols = np.zeros(256, np.int64)
        for fc in range(2):
            wqk_cols[fc * 128 : fc * 128 + 128] = (
                fc * DIM + (h0 + c128 // 64) * 64 + c128 % 64
            )
        wqk_b = W_qkv[:, wqk_cols].reshape(4, 128, 256).transpose(1, 0, 2)
        # wv: [p, kc, c] with c=(h*64+d)
        colsv = 2 * DIM + (h0 + c128 // 64) * 64 + c128 % 64
        wv_b = W_qkv[:, colsv].reshape(4, 128, 128).transpose(1, 0, 2)
        # wout: [(h,d), f] for the two heads stacked
        wout_b = W_out[h0 * 64 : (h0 + 2) * 64, :]
        tmain, tedge = _conv_templates(conv_w[h0 : h0 + 2, 0, :, 0])
        mb = np.where(mask[b], -2.0, NEG).astype(np.float32)
        in_maps.append(
            {
                "xT": np.ascontiguousarray(x[b].T).astype(bf),
                "wqk": np.ascontiguousarray(wqk_b).astype(bf),
                "wv": np.ascontiguousarray(wv_b).astype(bf),
                "wout": np.ascontiguousarray(wout_b).astype(bf),
                "tmain": tmain.astype(bf),
                "tedge": tedge.astype(bf),
                "mbias": np.ascontiguousarray(mb.reshape(16, 128).T),
            }
        )

    return in_maps


def _combine(results, mask, b_out):
    out = np.zeros((B, N, DIM), np.float32)
    for core in range(NCORES):
        out[core // 4] += np.asarray(results[core]["out"], np.float32)
    out += np.asarray(b_out, np.float32)[None, None, :]
    out *= np.asarray(mask)[:, :, None].astype(np.float32)
    return out


def kernel(x, mask, W_qkv, W_out, b_out, conv_w):
    from concourse.bass_utils import run_bass_kernel_spmd

    nc = _get_nc()
    in_maps = _make_in_maps(x, mask, W_qkv, W_out, conv_w)
    results = run_bass_kernel_spmd(nc, in_maps, list(range(NCORES))).results
    return _combine(results, mask, b_out)
